# revision 1
# baseline (speedup 1.0000x reference)
"""DSVT sparse-attention kernel for 8 Trainium2 NeuronCores.

Strategy: shard the set dimension (2048 sets -> 256 per core). Because each
layer's set_voxel_inds is a permutation of all N voxels, each core's 256 sets
cover exactly N/8 = 9216 distinct voxels, and the ENTIRE layer (attention +
LayerNorms + FFN + residuals) is per-voxel local once those rows are gathered.
Between layers the permutation changes, so cores exchange rows via an
all_to_all keyed by host-precomputed routing permutations (each voxel is
needed by exactly one core next layer). Block-residual rows ride the same
mechanism one boundary early and are threaded through.

The final layer's output shards are returned per-core and scattered back to
voxel order on the host.
"""
import numpy as np
import jax
import jax.numpy as jnp
from jax.sharding import Mesh, PartitionSpec as P
from jax.experimental.shard_map import shard_map

C = 192
H = 8
DH = C // H
FF = 384
SET = 36
NSET = 2048
N = NSET * SET
NBLK = 4
NLYR = 8
EPS = 1e-5
SCALE = np.float32(1.0 / np.sqrt(DH))
NC_ = 8
SPC = NSET // NC_          # sets per core = 256
RPC = N // NC_             # rows per core = 9216

WKEYS = (
    "in_proj_w", "in_proj_b", "out_proj_w", "out_proj_b",
    "lin1_w", "lin1_b", "lin2_w", "lin2_b",
    "norm1_w", "norm1_b", "norm2_w", "norm2_b",
    "enc_norm_w", "enc_norm_b", "blk_norm_w", "blk_norm_b",
)

_cache = {}


def _ln(x, w, b):
    m = jnp.mean(x, -1, keepdims=True)
    v = jnp.mean((x - m) ** 2, -1, keepdims=True)
    return (x - m) * jax.lax.rsqrt(v + EPS) * w + b


def _layer_rows(feat, pos_rows, mask_rows, l, W):
    """Per-core layer compute on gathered rows (everything voxel-local)."""
    in_w = W["in_proj_w"][l]
    in_b = W["in_proj_b"][l]
    q = feat + pos_rows
    qp = (q @ in_w[:C].T + in_b[:C]).reshape(SPC, SET, H, DH)
    kp = (q @ in_w[C:2 * C].T + in_b[C:2 * C]).reshape(SPC, SET, H, DH)
    vp = (feat @ in_w[2 * C:].T + in_b[2 * C:]).reshape(SPC, SET, H, DH)
    scores = jnp.einsum("sqhd,skhd->shqk", qp, kp) * SCALE
    scores = jnp.where(mask_rows[:, None, None, :], jnp.float32(-1e9), scores)
    attn = jax.nn.softmax(scores, axis=-1)
    o = jnp.einsum("shqk,skhd->sqhd", attn, vp).reshape(SPC * SET, C)
    o = o @ W["out_proj_w"][l].T + W["out_proj_b"][l]
    x = _ln(feat + o, W["norm1_w"][l], W["norm1_b"][l])
    ff = jax.nn.relu(x @ W["lin1_w"][l].T + W["lin1_b"][l]) @ W["lin2_w"][l].T + W["lin2_b"][l]
    return _ln(x + ff, W["norm2_w"][l], W["norm2_b"][l])


def _route(inds_by_layer, src_l, dst_l, B=None):
    """Routing for one exchange: rows produced in src_l order, consumed in
    dst_l order. Returns (send_idx [NC_, NC_, B], recv_idx [NC_, RPC], maxcnt)."""
    inv_src = np.empty(N, dtype=np.int64)
    inv_src[inds_by_layer[src_l]] = np.arange(N)
    dst_rows = inds_by_layer[dst_l].reshape(NC_, RPC)
    src_pos = inv_src[dst_rows]          # [NC_, RPC]
    src_core = src_pos // RPC
    src_local = src_pos % RPC
    maxcnt = 0
    for d in range(NC_):
        maxcnt = max(maxcnt, int(np.bincount(src_core[d], minlength=NC_).max()))
    if B is None:
        return int(maxcnt)
    send_idx = np.zeros((NC_, NC_, B), dtype=np.int32)
    recv_idx = np.zeros((NC_, RPC), dtype=np.int32)
    for d in range(NC_):
        sc = src_core[d]
        order = np.argsort(sc, kind="stable")
        counts = np.bincount(sc, minlength=NC_)
        starts = np.concatenate([[0], np.cumsum(counts)[:-1]])
        j_sorted = np.arange(RPC) - starts[sc[order]]
        ranks = np.empty(RPC, dtype=np.int64)
        ranks[order] = j_sorted
        send_idx[sc[order], d, j_sorted] = src_local[d][order].astype(np.int32)
        recv_idx[d] = (sc * B + ranks).astype(np.int32)
    return send_idx, recv_idx


def _build_jitted(B):
    mesh = Mesh(np.array(jax.devices()[:NC_]), ("c",))

    def inner(pillar, pos_embed, rows0, send_idx, recv_idx,
              res_send, res_recv, mask_rows, *wvals):
        rows0 = rows0[0]          # [NLYR, RPC]
        send_idx = send_idx[0]    # [7, NC_, B]
        recv_idx = recv_idx[0]    # [7, RPC]
        res_send = res_send[0]    # [3, NC_, B]
        res_recv = res_recv[0]    # [3, RPC]
        mask_rows = mask_rows[0]  # [NLYR, SPC, SET]
        W = dict(zip(WKEYS, wvals))

        def a2a(x):
            return jax.lax.all_to_all(x, "c", split_axis=0, concat_axis=0, tiled=True)

        feat = jnp.take(pillar, rows0[0], axis=0)
        res_rows = jnp.take(pillar, rows0[1], axis=0)   # block-0 residual, layer-1 keyed
        res_next = None
        out = None
        for l in range(NLYR):
            blk, s = l // 2, l % 2
            pos_rows = jnp.take(pos_embed[blk, s], rows0[l], axis=0)
            x2 = _layer_rows(feat, pos_rows, mask_rows[l], l, W)
            out = _ln(x2 + feat, W["enc_norm_w"][l], W["enc_norm_b"][l])
            if s == 1:
                out = _ln(res_rows + out, W["blk_norm_w"][blk], W["blk_norm_b"][blk])
            if l < NLYR - 1:
                send = jnp.take(out, send_idx[l].reshape(-1), axis=0)
                recv = a2a(send.reshape(NC_, B, C)).reshape(NC_ * B, C)
                feat = jnp.take(recv, recv_idx[l], axis=0)
                if s == 1:
                    ri = (l - 1) // 2
                    send2 = jnp.take(out, res_send[ri].reshape(-1), axis=0)
                    recv2 = a2a(send2.reshape(NC_, B, C)).reshape(NC_ * B, C)
                    res_next = jnp.take(recv2, res_recv[ri], axis=0)
                if s == 0 and l > 0:
                    res_rows = res_next
        return out

    fn = shard_map(
        inner, mesh=mesh,
        in_specs=(P(), P(), P("c"), P("c"), P("c"), P("c"), P("c"), P("c"))
        + (P(),) * len(WKEYS),
        out_specs=P("c"),
        check_rep=False,
    )
    return jax.jit(fn)


def kernel(**inputs):
    pillar = np.ascontiguousarray(inputs["pillar_features"], dtype=np.float32)
    inds0 = np.asarray(inputs["set_voxel_inds_tensor_shift_0"])
    inds1 = np.asarray(inputs["set_voxel_inds_tensor_shift_1"])
    mask0 = np.asarray(inputs["set_voxel_masks_tensor_shift_0"])
    mask1 = np.asarray(inputs["set_voxel_masks_tensor_shift_1"])
    pos_embed = np.ascontiguousarray(inputs["pos_embed_tensor"], dtype=np.float32)
    W = [np.ascontiguousarray(inputs[k], dtype=np.float32) for k in WKEYS]

    inds_by_layer = []
    masks_by_layer = []
    for l in range(NLYR):
        blk, s = l // 2, l % 2
        it = inds0 if blk % 2 == 0 else inds1
        mt = mask0 if blk % 2 == 0 else mask1
        inds_by_layer.append(np.asarray(it[s], dtype=np.int64).reshape(-1))
        masks_by_layer.append(np.asarray(mt[s]).astype(bool))

    pairs = [(l, l + 1) for l in range(NLYR - 1)] + [(l, l + 2) for l in (1, 3, 5)]
    B = max(_route(inds_by_layer, a, b) for a, b in pairs)

    send_list, recv_list = [], []
    for l in range(NLYR - 1):
        s, r = _route(inds_by_layer, l, l + 1, B)
        send_list.append(s)
        recv_list.append(r)
    res_send, res_recv = [], []
    for l in (1, 3, 5):
        s, r = _route(inds_by_layer, l, l + 2, B)
        res_send.append(s)
        res_recv.append(r)

    # per-core stacked tensors; axis 0 is the shard axis for P("c")
    rows0_all = np.stack([
        np.stack([inds_by_layer[l].reshape(NC_, RPC)[c] for l in range(NLYR)])
        for c in range(NC_)]).astype(np.int32)
    send_all = np.stack([np.stack([send_list[l][c] for l in range(NLYR - 1)])
                         for c in range(NC_)])
    recv_all = np.stack([np.stack([recv_list[l][c] for l in range(NLYR - 1)])
                         for c in range(NC_)])
    res_send_all = np.stack([np.stack([res_send[i][c] for i in range(3)])
                             for c in range(NC_)])
    res_recv_all = np.stack([np.stack([res_recv[i][c] for i in range(3)])
                             for c in range(NC_)])
    mask_all = np.stack([
        np.stack([masks_by_layer[l].reshape(NC_, SPC, SET)[c] for l in range(NLYR)])
        for c in range(NC_)])

    if B not in _cache:
        _cache[B] = _build_jitted(B)
    fn = _cache[B]

    out_shards = np.asarray(fn(
        pillar, pos_embed, rows0_all, send_all, recv_all,
        res_send_all, res_recv_all, mask_all, *W,
    ))
    full = np.empty((N, C), dtype=np.float32)
    full[inds_by_layer[NLYR - 1]] = out_shards.reshape(N, C)
    return full

